# revision 44
# baseline (speedup 1.0000x reference)
"""MoD (mixture-of-depths) Qwen2 block — Trainium2 Bass kernel, 8 NeuronCores.

Structure: only 256 of 2048 tokens per sequence are selected (gamma=0.125);
non-selected tokens are zeroed, so their K/V are exactly zero and contribute
exp(0)=1 per causally-visible zero key to each softmax denominator.  The block
collapses to dense compute over the gathered tokens plus a per-query count
correction (count_i = pos_i - rank_i); causality on gathered indices is plain
lower-triangular.

Parallelization: TP-way tensor parallel within groups of TP cores, sequences
data-parallel across the 8/TP groups.  bf16 matmuls (fp32 PSUM), feature-major
activations.  RMSNorm#1 is folded into the RoPE tables / a transposed V scale,
so QKV never waits on the norm.  bf16 AllReduce after wo per chunk; the down
projection runs "flipped" (stationary = m tiles, moving = w_down columns,
token-major output) feeding bf16 ReduceScatters split along tokens; the x1
residual is applied on the host from a dumped AR result.
"""
import numpy as np
import ml_dtypes

# ---- static problem config (hardcoded per spec) ----
B, S, D = 2, 2048, 2048
HQ, HKV, HD = 16, 8, 128
FF = 8192
GAMMA = 0.125
EPS = 1e-6
THETA = 10000.0
NCORES = 8

TP = 8                       # tensor-parallel degree (cores per group)
G = NCORES // TP             # sequence-parallel groups
NSEL = 256                   # selected tokens per sequence
TTOT = B * NSEL
T_G = TTOT // G              # tokens per group
TT_G = T_G // 128
S_G = B // G                 # sequences (= AR chunks) per group
TC = 256                     # tokens per chunk (one sequence)
NDT = D // 128
EQ = HQ * HD // TP
EQT = EQ // 128
EK = HKV * HD // TP
EKT = EK // 128
FG = FF // TP
FGT = FG // 128
RS_OUT = 128 // TP           # token rows per core per RS part

BF16 = ml_dtypes.bfloat16

_NC = None
_RUN_STATE = {}


def _pack_kxn(a):
    """[K, N] -> [128, (K/128)*N]; k-tile-major, full-width N chunks."""
    a = np.ascontiguousarray(a)
    K, N = a.shape
    return np.ascontiguousarray(
        a.reshape(K // 128, 128, N).transpose(1, 0, 2).reshape(128, -1))


def _pack_lhsT(a):
    """[K, M] -> [128, (M/128)*(K/128)*128]; cols of tile (mt, kt) start at
    (mt*KT + kt)*128."""
    a = np.ascontiguousarray(a)
    K, M = a.shape
    KT, MT = K // 128, M // 128
    return np.ascontiguousarray(
        a.reshape(KT, 128, MT, 128).transpose(1, 2, 0, 3).reshape(128, MT * KT * 128)
    )


def _build_nc():
    import concourse.mybir as mybir
    import concourse.tile as tile
    from concourse import bacc

    dt = mybir.dt
    f32, bf = dt.float32, dt.bfloat16
    Alu = mybir.AluOpType
    Act = mybir.ActivationFunctionType

    nc = bacc.Bacc("TRN2", target_bir_lowering=False, debug=False,
                   enable_asserts=False, num_devices=NCORES)

    def din(name, shape, dtype=f32):
        return nc.dram_tensor(name, list(shape), dtype, kind="ExternalInput").ap()

    # packed bf16 input: xT | wq | wk | wv | cosq|sinq|cosk|sink | pswap
    NB_XT = NDT * T_G
    NB_WQ = EQT * NDT * 128
    NB_WK = EKT * NDT * 128
    NB = NB_XT + NB_WQ + 2 * NB_WK + 4 * T_G + 128
    inB_in = din("inB", [128, NB], bf)
    # packed bf16 input 2: wo
    NC_WO = NDT * EQT * 128
    inC_in = din("inC", [128, NC_WO], bf)
    # packed f32 input: counts | cmask | ones | ident
    ND_ = TT_G + 3 * 128
    inD_in = din("inD", [128, ND_])
    wgT_in = din("wgT", [128, FGT * NDT * 128], bf)
    wuT_in = din("wuT", [128, FGT * NDT * 128], bf)
    wdT_in = din("wdT", [128, FGT * D], bf)
    WD_RES = TP >= 8

    out_ap = nc.dram_tensor("out_shard", [S_G * 2 * RS_OUT, D], bf,
                            kind="ExternalOutput").ap()
    x1s_out = nc.dram_tensor("x1s_out", [128, NDT * T_G], bf,
                             kind="ExternalOutput").ap()

    rg = [list(range(g * TP, (g + 1) * TP)) for g in range(G)]

    with tile.TileContext(nc) as tc:
        with (
            tc.tile_pool(name="const", bufs=1) as constp,
            tc.tile_pool(name="wres", bufs=1) as wres,
            tc.tile_pool(name="acts", bufs=1) as acts,
            tc.tile_pool(name="wslab", bufs=3) as wslab,
            tc.tile_pool(name="small", bufs=3) as small,
            tc.tile_pool(name="psum", bufs=5, space="PSUM") as psum,
            tc.tile_pool(name="dram", bufs=1, space="DRAM") as dram,
        ):
            # ---- packed input loads (few DMA issues; early regions first) ----
            inB = acts.tile([128, NB], bf, tag="inB")
            nc.sync.dma_start(inB[:, 0:NB_XT // 2], inB_in[:, 0:NB_XT // 2])
            nc.sync.dma_start(inB[:, NB_XT // 2:NB_XT + NB_WQ],
                              inB_in[:, NB_XT // 2:NB_XT + NB_WQ])
            nc.sync.dma_start(inB[:, NB_XT + NB_WQ:NB], inB_in[:, NB_XT + NB_WQ:NB])
            inD = constp.tile([128, ND_], f32, tag="inD")
            nc.sync.dma_start(inD[:], inD_in)
            inC = wres.tile([128, NC_WO], bf, tag="inC")
            nc.sync.dma_start(inC[:], inC_in)
            if WD_RES:
                wd_res = wres.tile([128, FGT * D], bf, tag="wd_res")
                nc.sync.dma_start(wd_res[:], wdT_in)

            xT = inB[:, 0:NB_XT].rearrange("p (a b) -> p a b", b=T_G)
            wq = inB[:, NB_XT:NB_XT + NB_WQ]
            wk = inB[:, NB_XT + NB_WQ:NB_XT + NB_WQ + NB_WK]
            wv = inB[:, NB_XT + NB_WQ + NB_WK:NB_XT + NB_WQ + 2 * NB_WK]
            cbase = NB_XT + NB_WQ + 2 * NB_WK
            cosq = inB[:, cbase:cbase + T_G]
            sinq = inB[:, cbase + T_G:cbase + 2 * T_G]
            cosk = inB[:, cbase + 2 * T_G:cbase + 3 * T_G]
            sink = inB[:, cbase + 3 * T_G:cbase + 4 * T_G]
            pswap = inB[:, cbase + 4 * T_G:cbase + 4 * T_G + 128]
            wo = inC[:, 0:NC_WO]
            counts = inD[:, 0:TT_G]
            cmask = inD[:, TT_G:TT_G + 128]
            ones = inD[:, TT_G + 128:TT_G + 256]
            ident = inD[:, TT_G + 256:TT_G + 384]
            eps_sb = constp.tile([1, 1], f32, tag="eps")
            nc.vector.memset(eps_sb[:], EPS)
            ones_bf = constp.tile([128, 1], bf, tag="ones_bf")
            nc.vector.memset(ones_bf[:], 1.0)

            # ---- rmsnorm stats: rbc [128, n] psum with rows = rinv[t] ----
            def rms_stats(x3, n, ncols):
                msum = psum.tile([1, n], f32, tag="msum", bufs=1)
                for hh in range(2):
                    hsl = slice(hh * NDT // 2, (hh + 1) * NDT // 2)
                    sqa = small.tile([128, NDT // 2, n], bf, tag="sqa", bufs=2)
                    nc.vector.tensor_mul(sqa[:], x3[:, hsl, ncols],
                                         x3[:, hsl, ncols])
                    for dd in range(NDT // 2):
                        dti = hh * NDT // 2 + dd
                        nc.tensor.matmul(msum[:], ones_bf[:], sqa[:, dd, :],
                                         start=(dti == 0),
                                         stop=(dti == NDT - 1))
                rinv = small.tile([1, n], f32, tag="rinv")
                nc.scalar.activation(rinv[:], msum[:], Act.Abs_reciprocal_sqrt,
                                     bias=eps_sb[:], scale=1.0 / D)
                rbc_ps = psum.tile([128, n], f32, tag="rbc", bufs=1)
                nc.tensor.matmul(rbc_ps[:], ones[0:1], rinv[:], start=True,
                                 stop=True)
                rbc = small.tile([128, n], bf, tag="rbc_sb", bufs=2)
                nc.scalar.copy(rbc[:], rbc_ps[:])
                return rbc, rinv

            # ---- norm1 stats (norm folded into rope tables / V scale) ----
            rbc1, rinv1 = rms_stats(xT, T_G, slice(0, T_G))
            rinv_t = small.tile([128, TT_G], f32, tag="rinv_t", bufs=1)
            for tt in range(TT_G):
                rtp = psum.tile([128, 1], f32, tag="ps")
                nc.tensor.transpose(rtp[:], rinv1[0:1, tt * 128:(tt + 1) * 128],
                                    ident[0:1, 0:1])
                nc.vector.tensor_copy(rinv_t[:, tt:tt + 1], rtp[:])
            cq_s = acts.tile([128, T_G], bf, tag="cq_s")
            sq_s = acts.tile([128, T_G], bf, tag="sq_s")
            ck_s = acts.tile([128, T_G], bf, tag="ck_s")
            sk_s = acts.tile([128, T_G], bf, tag="sk_s")
            nc.vector.tensor_mul(cq_s[:], cosq, rbc1[:])
            nc.vector.tensor_mul(sq_s[:], sinq, rbc1[:])
            nc.vector.tensor_mul(ck_s[:], cosk, rbc1[:])
            nc.vector.tensor_mul(sk_s[:], sink, rbc1[:])

            # ---- QKV (all tokens, N=T_G) + full-width RoPE ----
            q_sb = acts.tile([128, EQT, T_G], bf, tag="q")
            k_sb = acts.tile([128, EKT, T_G], bf, tag="k")
            v_sb = acts.tile([128, EKT, TT_G, 128], bf, tag="v")
            ctx = acts.tile([128, EQT, T_G], bf, tag="ctx")

            def rope(ps, cos_t, sin_t, out2):
                raw = small.tile([128, T_G], bf, tag="rope_raw")
                nc.scalar.copy(raw[:], ps[:])
                rot = psum.tile([128, T_G], f32, tag="ps")
                nc.tensor.matmul(rot[:], pswap, raw[:], start=True, stop=True)
                t1 = small.tile([128, T_G], f32, tag="rope_t1", bufs=2)
                nc.vector.tensor_mul(t1[:], raw[:], cos_t)
                t2 = small.tile([128, T_G], f32, tag="rope_t2", bufs=2)
                nc.vector.tensor_mul(t2[:], rot[:], sin_t)
                nc.vector.tensor_add(out2, t1[:], t2[:])

            for et in range(EQT):
                ps = psum.tile([128, T_G], f32, tag="ps")
                for dti in range(NDT):
                    c0 = (et * NDT + dti) * 128
                    nc.tensor.matmul(ps[:], wq[:, c0:c0 + 128], xT[:, dti, :],
                                     start=(dti == 0), stop=(dti == NDT - 1))
                rope(ps, cq_s[:], sq_s[:], q_sb[:, et, :])
            for et in range(EKT):
                ps = psum.tile([128, T_G], f32, tag="ps")
                for dti in range(NDT):
                    c0 = (et * NDT + dti) * 128
                    nc.tensor.matmul(ps[:], wk[:, c0:c0 + 128], xT[:, dti, :],
                                     start=(dti == 0), stop=(dti == NDT - 1))
                rope(ps, ck_s[:], sk_s[:], k_sb[:, et, :])
            for kh in range(EKT):
                for tt in range(TT_G):
                    vp = psum.tile([128, 128], f32, tag="ps")
                    for dti in range(NDT):
                        c0 = (kh * NDT + dti) * 128
                        nc.tensor.matmul(
                            vp[:], xT[:, dti, tt * 128:(tt + 1) * 128],
                            wv[:, c0:c0 + 128],
                            start=(dti == 0), stop=(dti == NDT - 1))
                    nc.vector.tensor_scalar_mul(v_sb[:, kh, tt, :], vp[:],
                                                rinv_t[:, tt:tt + 1])

            x1_ch = []
            h2_ch = []
            for s in range(S_G):
                base = s * 256
                cols = slice(base, base + 256)
                # ---- attention per local q-head ----
                for h in range(EQT):
                    kh = h // 2
                    s0 = psum.tile([128, 128], f32, tag="ps")
                    nc.tensor.matmul(s0[:], q_sb[:, h, base:base + 128],
                                     k_sb[:, kh, base:base + 128],
                                     start=True, stop=True)
                    s1 = psum.tile([128, 256], f32, tag="ps")
                    nc.tensor.matmul(s1[:], q_sb[:, h, base + 128:base + 256],
                                     k_sb[:, kh, base:base + 256],
                                     start=True, stop=True)
                    nc.vector.tensor_add(s0[:], s0[:], cmask)
                    nc.vector.tensor_add(s1[:, 128:256], s1[:, 128:256], cmask)
                    e0 = small.tile([128, 128], f32, tag="e0")
                    e1 = small.tile([128, 256], f32, tag="e1", bufs=2)
                    den = small.tile([128, 2], f32, tag="den")
                    nc.scalar.activation(e0[:], s0[:], Act.Exp,
                                         accum_out=den[:, 0:1])
                    nc.scalar.activation(e1[:], s1[:], Act.Exp,
                                         accum_out=den[:, 1:2])
                    qt0 = 2 * s
                    nc.vector.tensor_add(den[:], den[:], counts[:, qt0:qt0 + 2])
                    rr = small.tile([128, 2], f32, tag="rr")
                    nc.vector.reciprocal(rr[:], den[:])
                    a0 = small.tile([128, 128], f32, tag="a0")
                    a1 = small.tile([128, 256], f32, tag="a1", bufs=2)
                    nc.vector.tensor_scalar_mul(a0[:], e0[:], rr[:, 0:1])
                    nc.vector.tensor_scalar_mul(a1[:], e1[:], rr[:, 1:2])
                    atb = small.tile([128, 3, 128], bf, tag="atb", bufs=2)
                    for i, ablk in enumerate((a0[:], a1[:, 0:128],
                                              a1[:, 128:256])):
                        atp = psum.tile([128, 128], f32, tag="ps")
                        nc.tensor.transpose(atp[:], ablk, ident)
                        nc.scalar.copy(atb[:, i, :], atp[:])
                    c0p = psum.tile([128, 128], f32, tag="ps")
                    nc.tensor.matmul(c0p[:], v_sb[:, kh, 2 * s, :], atb[:, 0, :],
                                     start=True, stop=True)
                    c1p = psum.tile([128, 128], f32, tag="ps")
                    nc.tensor.matmul(c1p[:], v_sb[:, kh, 2 * s, :], atb[:, 1, :],
                                     start=True, stop=False)
                    nc.tensor.matmul(c1p[:], v_sb[:, kh, 2 * s + 1, :],
                                     atb[:, 2, :], start=False, stop=True)
                    nc.scalar.copy(ctx[:, h, base:base + 128], c0p[:])
                    nc.scalar.copy(ctx[:, h, base + 128:base + 256], c1p[:])

                # ---- wo partial (this chunk) + bf16 AllReduce ----
                ch = s
                x1p = acts.tile([128, NDT, TC], bf, tag="x1p", bufs=1)
                for fp in range(NDT // 2):
                    ps = psum.tile([128, 2, TC], f32, tag="ps")
                    for sub in range(2):
                        ft = fp * 2 + sub
                        for et in range(EQT):
                            c0 = (ft * EQT + et) * 128
                            nc.tensor.matmul(ps[:, sub, :], wo[:, c0:c0 + 128],
                                             ctx[:, et, cols],
                                             start=(et == 0),
                                             stop=(et == EQT - 1))
                    if fp % 2 == 0:
                        nc.vector.tensor_copy(x1p[:, fp * 2:fp * 2 + 2, :], ps[:])
                    else:
                        nc.scalar.copy(x1p[:, fp * 2:fp * 2 + 2, :], ps[:])
                ar_in = dram.tile([128, NDT * TC], bf, tag=f"ar_in{ch}",
                                  name=f"ar_in{ch}")
                ar_out = dram.tile(
                    [128, NDT * TC], bf, tag=f"ar_out{ch}",
                    name=f"ar_out{ch}",
                    addr_space="Shared" if TP > 4 else "Local")
                nc.sync.dma_start(ar_in[:], x1p[:])
                nc.gpsimd.collective_compute(
                    "AllReduce", mybir.AluOpType.add, replica_groups=rg,
                    ins=[ar_in.opt()], outs=[ar_out.opt()])
                x1s = acts.tile([128, NDT, TC], bf, tag="x1s",
                                name=f"x1s_{ch}", bufs=1)
                nc.sync.dma_start(
                    x1s[:], ar_out[:].rearrange("p (a b) -> p a b", b=TC))
                nc.sync.dma_start(
                    x1s_out.rearrange("p (a b) -> p a b", b=T_G)[:, :, cols],
                    ar_out[:].rearrange("p (a b) -> p a b", b=TC))
                x1 = acts.tile([128, NDT, TC], bf, tag=f"x1_{ch}",
                               name=f"x1_{ch}")
                nc.vector.tensor_add(x1[:], x1s[:], xT[:, :, cols])
                x1_ch.append(x1)
                # norm2 for this chunk
                rbc2, _ = rms_stats(x1, TC, slice(0, TC))
                h2 = acts.tile([128, NDT, TC], bf, tag=f"h2_{ch}",
                               name=f"h2_{ch}")
                for qd in range(4):
                    dsl_ = slice(qd * 4, qd * 4 + 4)
                    nc.vector.tensor_tensor(
                        h2[:, dsl_, :], x1[:, dsl_, :],
                        rbc2[:, None, :].broadcast_to([128, 4, TC]), Alu.mult)
                h2_ch.append(h2)

            # ---- MLP per chunk: gate/up -> down(flipped) -> ReduceScatter ----
            m_ch = [acts.tile([128, FGT, TC], bf, tag=f"m_{ch}", name=f"m_{ch}")
                    for ch in range(S_G)]
            for ch in range(S_G):
                for ft in range(FGT):
                    gsl = wslab.tile([128, NDT * 128], bf, tag="wslab")
                    nc.gpsimd.dma_start(
                        gsl[:], wgT_in[:, ft * NDT * 128:(ft + 1) * NDT * 128])
                    usl = wslab.tile([128, NDT * 128], bf, tag="wslab")
                    nc.gpsimd.dma_start(
                        usl[:], wuT_in[:, ft * NDT * 128:(ft + 1) * NDT * 128])
                    gp = psum.tile([128, TC], f32, tag="ps")
                    up = psum.tile([128, TC], f32, tag="ps")
                    for dti in range(NDT):
                        nc.tensor.matmul(gp[:], gsl[:, dti * 128:(dti + 1) * 128],
                                         h2_ch[ch][:, dti, :],
                                         start=(dti == 0), stop=(dti == NDT - 1))
                    for dti in range(NDT):
                        nc.tensor.matmul(up[:], usl[:, dti * 128:(dti + 1) * 128],
                                         h2_ch[ch][:, dti, :],
                                         start=(dti == 0), stop=(dti == NDT - 1))
                    gs = small.tile([128, TC], f32, tag="gs", bufs=2)
                    nc.scalar.activation(gs[:], gp[:], Act.Silu)
                    nc.vector.tensor_mul(m_ch[ch][:, ft, :], gs[:], up[:])

                # down flipped: lhsT = m tiles, rhs = w_down column slabs
                for part in range(2):
                    rs_in = dram.tile([128, D], bf, tag=f"rs_in{ch}_{part}",
                                      name=f"rs_in{ch}_{part}")
                    tsl = slice(part * 128, part * 128 + 128)
                    for och in range(4):
                        if WD_RES:
                            dsl = wd_res.rearrange(
                                "p (a b) -> p a b", b=D)[:, :, och * 512:
                                                         (och + 1) * 512]
                        else:
                            dslt = wslab.tile([128, FGT, 512], bf,
                                              tag="wdslab", bufs=2)
                            nc.gpsimd.dma_start(
                                dslt[:],
                                wdT_in.rearrange("p (a b) -> p a b", b=D)
                                [:, :, och * 512:(och + 1) * 512])
                            dsl = dslt[:]
                        ps = psum.tile([128, 512], f32, tag="ps")
                        for ft in range(FGT):
                            nc.tensor.matmul(
                                ps[:], m_ch[ch][:, ft, tsl],
                                dsl[:, ft, :],
                                start=(ft == 0), stop=(ft == FGT - 1))
                        dr = small.tile([128, 512], bf, tag="x2dr", bufs=2)
                        if och % 2 == 0:
                            nc.vector.tensor_copy(dr[:], ps[:])
                        else:
                            nc.scalar.copy(dr[:], ps[:])
                        nc.sync.dma_start(
                            rs_in[:, och * 512:(och + 1) * 512], dr[:])
                    rs_out = dram.tile([128 // TP, D], bf,
                                       tag=f"rs_out{ch}_{part}",
                                       name=f"rs_out{ch}_{part}")
                    nc.gpsimd.collective_compute(
                        "ReduceScatter", mybir.AluOpType.add, replica_groups=rg,
                        ins=[rs_in.opt()], outs=[rs_out.opt()])
                    orow = (ch * 2 + part) * RS_OUT
                    nc.sync.dma_start(out_ap[orow:orow + RS_OUT, :], rs_out[:])

    nc.compile()
    return nc


def _host_prep(hidden_states, router_w, wq, wk, wv, wo, w_gate, w_up, w_down,
               ln1_w, ln2_w):
    x0 = np.asarray(hidden_states, np.float32)
    router_w = np.asarray(router_w, np.float32)
    rw = (x0.reshape(B * S, D) @ router_w.reshape(D)).reshape(B, S)
    k_cap = max(1, int(GAMMA * S))
    sel_idx, counts, rw_sel, xsel = [], [], [], []
    for b in range(B):
        thr = np.partition(rw[b], S - k_cap)[S - k_cap]
        idx = np.nonzero(rw[b] >= thr)[0]
        sel_idx.append(idx)
        counts.append((idx - np.arange(len(idx))).astype(np.float32))
        rw_sel.append(rw[b, idx])
        xsel.append(x0[b, idx])

    inv = 1.0 / (THETA ** (np.arange(0, HD, 2, dtype=np.float32) / HD))
    sgn = np.concatenate([-np.ones(64, np.float32), np.ones(64, np.float32)])
    cos_l, sin_l = [], []
    for b in range(B):
        fr = sel_idx[b].astype(np.float32)[:, None] * inv[None, :]
        emb = np.concatenate([fr, fr], axis=1)
        cos_l.append(np.cos(emb).T)
        sin_l.append((np.sin(emb) * sgn[None, :]).T)

    scale = np.float32(1.0 / np.sqrt(HD))
    xsel_all = np.concatenate(xsel, axis=0)             # [512, 2048]
    cos_all = np.concatenate(cos_l, axis=1)
    sin_all = np.concatenate(sin_l, axis=1)
    counts_all = np.concatenate(counts)

    cmask = np.triu(np.full((128, 128), -60000.0, np.float32), 1)
    pswap = np.zeros((128, 128), np.float32)
    pswap[(np.arange(128) + 64) % 128, np.arange(128)] = 1.0
    ones = np.ones((128, 128), np.float32)
    ident = np.eye(128, dtype=np.float32)

    ln1 = np.asarray(ln1_w, np.float32)
    ln2 = np.asarray(ln2_w, np.float32)
    wq_f = np.asarray(wq, np.float32) * ln1[None, :]
    wk_f = np.asarray(wk, np.float32) * ln1[None, :]
    wv_f = np.asarray(wv, np.float32) * ln1[None, :]
    wo_f = np.asarray(wo, np.float32)
    wg_f = np.asarray(w_gate, np.float32) * ln2[None, :]
    wu_f = np.asarray(w_up, np.float32) * ln2[None, :]
    wd_f = np.asarray(w_down, np.float32)

    in_maps = []
    for c in range(NCORES):
        g, r = c // TP, c % TP
        tokens = np.arange(g * T_G, (g + 1) * T_G)
        cos_g = cos_all[:, tokens]
        sin_g = sin_all[:, tokens]
        counts_g = counts_all[tokens]
        xsel_g = xsel_all[tokens]
        inB = np.concatenate([
            _pack_kxn(xsel_g.T.astype(np.float32)).astype(BF16),
            _pack_lhsT(wq_f[r * EQ:(r + 1) * EQ].T).astype(BF16),
            _pack_lhsT(wk_f[r * EK:(r + 1) * EK].T).astype(BF16),
            _pack_lhsT(wv_f[r * EK:(r + 1) * EK].T).astype(BF16),
            (cos_g * scale).astype(BF16),
            (sin_g * scale).astype(BF16),
            cos_g.astype(BF16),
            sin_g.astype(BF16),
            pswap.astype(BF16),
        ], axis=1)
        inC = _pack_lhsT(wo_f.T[r * EQ:(r + 1) * EQ]).astype(BF16)
        inD = np.concatenate([
            np.ascontiguousarray(
                counts_g.reshape(TT_G, 128).T).astype(np.float32),
            cmask, ones, ident,
        ], axis=1)
        m = {
            "inB": np.ascontiguousarray(inB),
            "inC": np.ascontiguousarray(inC),
            "inD": np.ascontiguousarray(inD),
            "wgT": _pack_lhsT(wg_f[r * FG:(r + 1) * FG].T).astype(BF16),
            "wuT": _pack_lhsT(wu_f[r * FG:(r + 1) * FG].T).astype(BF16),
            "wdT": _pack_kxn(wd_f.T[r * FG:(r + 1) * FG]).astype(BF16),
        }
        in_maps.append(m)
    return x0, sel_idx, rw_sel, xsel_all, in_maps


def kernel(hidden_states, router_w, wq, bq, wk, bk, wv, bv, wo,
           w_gate, w_up, w_down, ln1_w, ln2_w):
    global _NC
    from concourse import bass_utils

    x0, sel_idx, rw_sel, xsel_all, in_maps = _host_prep(
        hidden_states, router_w, wq, wk, wv, wo, w_gate, w_up, w_down,
        ln1_w, ln2_w)

    if _NC is None:
        _NC = _build_nc()

    res = bass_utils.run_bass_kernel_spmd(
        _NC, in_maps, core_ids=list(range(NCORES)),
        **_RUN_STATE.get("run_kwargs", {}))
    _RUN_STATE["last_results"] = res

    # x1 (pre-MLP residual stream) from dumped AR results, one core per group
    x1sT = np.empty((D, TTOT), np.float32)
    for g in range(G):
        xv = res.results[g * TP]["x1s_out"].astype(np.float32)
        xv = xv.reshape(128, NDT, T_G).transpose(1, 0, 2).reshape(D, T_G)
        x1sT[:, g * T_G:(g + 1) * T_G] = xv
    x1_full = x1sT.T + xsel_all                         # [512, 2048]

    # x2 (mlp output) from token-major RS shards
    x2 = np.empty((TTOT, D), np.float32)
    for c in range(NCORES):
        g, r = c // TP, c % TP
        sh = res.results[c]["out_shard"].astype(np.float32)
        for ch in range(S_G):
            for part in range(2):
                orow = (ch * 2 + part) * RS_OUT
                t0 = g * T_G + ch * 256 + part * 128 + r * RS_OUT
                x2[t0:t0 + RS_OUT, :] = sh[orow:orow + RS_OUT, :]
    block_out = x1_full + x2

    final = x0.copy()
    for b in range(B):
        rows = block_out[b * NSEL:(b + 1) * NSEL] * rw_sel[b][:, None]
        final[b, sel_idx[b]] = rows
    return final.astype(np.float32)


# revision 46
# speedup vs baseline: 1.0160x; 1.0160x over previous
"""MoD (mixture-of-depths) Qwen2 block — Trainium2 Bass kernel, 8 NeuronCores.

Structure: only 256 of 2048 tokens per sequence are selected (gamma=0.125);
non-selected tokens are zeroed, so their K/V are exactly zero and contribute
exp(0)=1 per causally-visible zero key to each softmax denominator.  The block
collapses to dense compute over the gathered tokens plus a per-query count
correction (count_i = pos_i - rank_i); causality on gathered indices is plain
lower-triangular.

Parallelization: TP-way tensor parallel within groups of TP cores, sequences
data-parallel across the 8/TP groups.  bf16 matmuls (fp32 PSUM), feature-major
activations.  RMSNorm#1 is folded into the RoPE tables / a transposed V scale,
so QKV never waits on the norm.  bf16 AllReduce after wo per chunk; the down
projection runs "flipped" (stationary = m tiles, moving = w_down columns,
token-major output) feeding bf16 ReduceScatters split along tokens; the x1
residual is applied on the host from a dumped AR result.
"""
import numpy as np
import ml_dtypes

# ---- static problem config (hardcoded per spec) ----
B, S, D = 2, 2048, 2048
HQ, HKV, HD = 16, 8, 128
FF = 8192
GAMMA = 0.125
EPS = 1e-6
THETA = 10000.0
NCORES = 8

TP = 8                       # tensor-parallel degree (cores per group)
G = NCORES // TP             # sequence-parallel groups
NSEL = 256                   # selected tokens per sequence
TTOT = B * NSEL
T_G = TTOT // G              # tokens per group
TT_G = T_G // 128
S_G = B // G                 # sequences (= AR chunks) per group
TC = 256                     # tokens per chunk (one sequence)
NDT = D // 128
EQ = HQ * HD // TP
EQT = EQ // 128
EK = HKV * HD // TP
EKT = EK // 128
FG = FF // TP
FGT = FG // 128
RS_OUT = 128 // TP           # token rows per core per RS part

BF16 = ml_dtypes.bfloat16

_NC = None
_RUN_STATE = {}


def _pack_kxn(a):
    """[K, N] -> [128, (K/128)*N]; k-tile-major, full-width N chunks."""
    a = np.ascontiguousarray(a)
    K, N = a.shape
    return np.ascontiguousarray(
        a.reshape(K // 128, 128, N).transpose(1, 0, 2).reshape(128, -1))


def _pack_lhsT(a):
    """[K, M] -> [128, (M/128)*(K/128)*128]; cols of tile (mt, kt) start at
    (mt*KT + kt)*128."""
    a = np.ascontiguousarray(a)
    K, M = a.shape
    KT, MT = K // 128, M // 128
    return np.ascontiguousarray(
        a.reshape(KT, 128, MT, 128).transpose(1, 2, 0, 3).reshape(128, MT * KT * 128)
    )


def _build_nc():
    import concourse.mybir as mybir
    import concourse.tile as tile
    from concourse import bacc

    dt = mybir.dt
    f32, bf = dt.float32, dt.bfloat16
    Alu = mybir.AluOpType
    Act = mybir.ActivationFunctionType

    nc = bacc.Bacc("TRN2", target_bir_lowering=False, debug=False,
                   enable_asserts=False, num_devices=NCORES)

    def din(name, shape, dtype=f32):
        return nc.dram_tensor(name, list(shape), dtype, kind="ExternalInput").ap()

    # packed bf16 input: xT | wq | wk | wv | cosq|sinq|cosk|sink | pswap
    NB_XT = NDT * T_G
    NB_WQ = EQT * NDT * 128
    NB_WK = EKT * NDT * 128
    NB = NB_XT + NB_WQ + 2 * NB_WK + 4 * T_G + 128
    inB_in = din("inB", [128, NB], bf)
    # packed bf16 input 2: wo
    NC_WO = NDT * EQT * 128
    inC_in = din("inC", [128, NC_WO], bf)
    # packed f32 input: counts | cmask | ones | ident
    ND_ = TT_G + 3 * 128
    inD_in = din("inD", [128, ND_])
    wgT_in = din("wgT", [128, FGT * NDT * 128], bf)
    wuT_in = din("wuT", [128, FGT * NDT * 128], bf)
    wdT_in = din("wdT", [128, FGT * D], bf)
    WD_RES = TP >= 8

    out_ap = nc.dram_tensor("out_shard", [S_G * 2 * RS_OUT, D], bf,
                            kind="ExternalOutput").ap()
    x1s_out = nc.dram_tensor("x1s_out", [128, NDT * T_G], bf,
                             kind="ExternalOutput").ap()

    rg = [list(range(g * TP, (g + 1) * TP)) for g in range(G)]

    with tile.TileContext(nc) as tc:
        with (
            tc.tile_pool(name="const", bufs=1) as constp,
            tc.tile_pool(name="wres", bufs=1) as wres,
            tc.tile_pool(name="acts", bufs=1) as acts,
            tc.tile_pool(name="wslab", bufs=3) as wslab,
            tc.tile_pool(name="small", bufs=3) as small,
            tc.tile_pool(name="psum", bufs=5, space="PSUM") as psum,
            tc.tile_pool(name="dram", bufs=1, space="DRAM") as dram,
        ):
            # ---- packed input loads (few DMA issues; early regions first) ----
            inB = acts.tile([128, NB], bf, tag="inB")
            nc.sync.dma_start(inB[:, 0:NB_XT // 2], inB_in[:, 0:NB_XT // 2])
            nc.sync.dma_start(inB[:, NB_XT // 2:NB_XT + NB_WQ],
                              inB_in[:, NB_XT // 2:NB_XT + NB_WQ])
            nc.sync.dma_start(inB[:, NB_XT + NB_WQ:NB], inB_in[:, NB_XT + NB_WQ:NB])
            inD = constp.tile([128, ND_], f32, tag="inD")
            nc.sync.dma_start(inD[:], inD_in)
            inC = wres.tile([128, NC_WO], bf, tag="inC")
            nc.sync.dma_start(inC[:], inC_in)
            if WD_RES:
                wd_res = wres.tile([128, FGT * D], bf, tag="wd_res")
                nc.sync.dma_start(wd_res[:], wdT_in)

            xT = inB[:, 0:NB_XT].rearrange("p (a b) -> p a b", b=T_G)
            wq = inB[:, NB_XT:NB_XT + NB_WQ]
            wk = inB[:, NB_XT + NB_WQ:NB_XT + NB_WQ + NB_WK]
            wv = inB[:, NB_XT + NB_WQ + NB_WK:NB_XT + NB_WQ + 2 * NB_WK]
            cbase = NB_XT + NB_WQ + 2 * NB_WK
            cosq = inB[:, cbase:cbase + T_G]
            sinq = inB[:, cbase + T_G:cbase + 2 * T_G]
            cosk = inB[:, cbase + 2 * T_G:cbase + 3 * T_G]
            sink = inB[:, cbase + 3 * T_G:cbase + 4 * T_G]
            pswap = inB[:, cbase + 4 * T_G:cbase + 4 * T_G + 128]
            wo = inC[:, 0:NC_WO]
            counts = inD[:, 0:TT_G]
            cmask = inD[:, TT_G:TT_G + 128]
            ones = inD[:, TT_G + 128:TT_G + 256]
            ident = inD[:, TT_G + 256:TT_G + 384]
            eps_sb = constp.tile([1, 1], f32, tag="eps")
            nc.vector.memset(eps_sb[:], EPS)
            ones_bf = constp.tile([128, 1], bf, tag="ones_bf")
            nc.vector.memset(ones_bf[:], 1.0)

            # ---- rmsnorm stats: rbc [128, n] psum with rows = rinv[t] ----
            def rms_stats(x3, n, ncols):
                msum = psum.tile([1, n], f32, tag="msum", bufs=1)
                for hh in range(2):
                    hsl = slice(hh * NDT // 2, (hh + 1) * NDT // 2)
                    sqa = small.tile([128, NDT // 2, n], bf, tag="sqa", bufs=2)
                    nc.vector.tensor_mul(sqa[:], x3[:, hsl, ncols],
                                         x3[:, hsl, ncols])
                    for dd in range(NDT // 2):
                        dti = hh * NDT // 2 + dd
                        nc.tensor.matmul(msum[:], ones_bf[:], sqa[:, dd, :],
                                         start=(dti == 0),
                                         stop=(dti == NDT - 1))
                rinv = small.tile([1, n], f32, tag="rinv")
                nc.scalar.activation(rinv[:], msum[:], Act.Abs_reciprocal_sqrt,
                                     bias=eps_sb[:], scale=1.0 / D)
                rbc_ps = psum.tile([128, n], f32, tag="rbc", bufs=1)
                nc.tensor.matmul(rbc_ps[:], ones[0:1], rinv[:], start=True,
                                 stop=True)
                rbc = small.tile([128, n], bf, tag="rbc_sb", bufs=2)
                nc.scalar.copy(rbc[:], rbc_ps[:])
                return rbc, rinv

            # ---- norm1 stats (norm folded into rope tables / V scale) ----
            rbc1, rinv1 = rms_stats(xT, T_G, slice(0, T_G))
            rinv_t = small.tile([128, TT_G], f32, tag="rinv_t", bufs=1)
            for tt in range(TT_G):
                rtp = psum.tile([128, 1], f32, tag="ps")
                nc.tensor.transpose(rtp[:], rinv1[0:1, tt * 128:(tt + 1) * 128],
                                    ident[0:1, 0:1])
                nc.vector.tensor_copy(rinv_t[:, tt:tt + 1], rtp[:])
            cq_s = acts.tile([128, T_G], bf, tag="cq_s")
            sq_s = acts.tile([128, T_G], bf, tag="sq_s")
            ck_s = acts.tile([128, T_G], bf, tag="ck_s")
            sk_s = acts.tile([128, T_G], bf, tag="sk_s")
            nc.vector.tensor_mul(cq_s[:], cosq, rbc1[:])
            nc.vector.tensor_mul(sq_s[:], sinq, rbc1[:])
            nc.vector.tensor_mul(ck_s[:], cosk, rbc1[:])
            nc.vector.tensor_mul(sk_s[:], sink, rbc1[:])

            # ---- QKV (all tokens, N=T_G) + full-width RoPE ----
            q_sb = acts.tile([128, EQT, T_G], bf, tag="q")
            k_sb = acts.tile([128, EKT, T_G], bf, tag="k")
            v_sb = acts.tile([128, EKT, TT_G, 128], bf, tag="v")
            ctx = acts.tile([128, EQT, T_G], bf, tag="ctx")

            def rope(ps, cos_t, sin_t, out2):
                raw = small.tile([128, T_G], bf, tag="rope_raw")
                nc.scalar.copy(raw[:], ps[:])
                rot = psum.tile([128, T_G], f32, tag="ps")
                nc.tensor.matmul(rot[:], pswap, raw[:], start=True, stop=True)
                t1 = small.tile([128, T_G], f32, tag="rope_t1", bufs=2)
                nc.vector.tensor_mul(t1[:], raw[:], cos_t)
                t2 = small.tile([128, T_G], f32, tag="rope_t2", bufs=2)
                nc.vector.tensor_mul(t2[:], rot[:], sin_t)
                nc.vector.tensor_add(out2, t1[:], t2[:])

            for et in range(EQT):
                ps = psum.tile([128, T_G], f32, tag="ps")
                for dti in range(NDT):
                    c0 = (et * NDT + dti) * 128
                    nc.tensor.matmul(ps[:], wq[:, c0:c0 + 128], xT[:, dti, :],
                                     start=(dti == 0), stop=(dti == NDT - 1))
                rope(ps, cq_s[:], sq_s[:], q_sb[:, et, :])
            for et in range(EKT):
                ps = psum.tile([128, T_G], f32, tag="ps")
                for dti in range(NDT):
                    c0 = (et * NDT + dti) * 128
                    nc.tensor.matmul(ps[:], wk[:, c0:c0 + 128], xT[:, dti, :],
                                     start=(dti == 0), stop=(dti == NDT - 1))
                rope(ps, ck_s[:], sk_s[:], k_sb[:, et, :])
            for kh in range(EKT):
                for tt in range(TT_G):
                    vp = psum.tile([128, 128], f32, tag="ps")
                    for dti in range(NDT):
                        c0 = (kh * NDT + dti) * 128
                        nc.tensor.matmul(
                            vp[:], xT[:, dti, tt * 128:(tt + 1) * 128],
                            wv[:, c0:c0 + 128],
                            start=(dti == 0), stop=(dti == NDT - 1))
                    nc.vector.tensor_scalar_mul(v_sb[:, kh, tt, :], vp[:],
                                                rinv_t[:, tt:tt + 1])

            x1_ch = []
            h2_ch = []
            for s in range(S_G):
                base = s * 256
                cols = slice(base, base + 256)
                # ---- attention per local q-head ----
                for h in range(EQT):
                    kh = h // 2
                    s0 = psum.tile([128, 128], f32, tag="ps")
                    nc.tensor.matmul(s0[:], q_sb[:, h, base:base + 128],
                                     k_sb[:, kh, base:base + 128],
                                     start=True, stop=True)
                    s1 = psum.tile([128, 256], f32, tag="ps")
                    nc.tensor.matmul(s1[:], q_sb[:, h, base + 128:base + 256],
                                     k_sb[:, kh, base:base + 256],
                                     start=True, stop=True)
                    nc.vector.tensor_add(s0[:], s0[:], cmask)
                    nc.vector.tensor_add(s1[:, 128:256], s1[:, 128:256], cmask)
                    e0 = small.tile([128, 128], f32, tag="e0")
                    e1 = small.tile([128, 256], f32, tag="e1", bufs=2)
                    den = small.tile([128, 2], f32, tag="den")
                    nc.scalar.activation(e0[:], s0[:], Act.Exp,
                                         accum_out=den[:, 0:1])
                    nc.scalar.activation(e1[:], s1[:], Act.Exp,
                                         accum_out=den[:, 1:2])
                    qt0 = 2 * s
                    nc.vector.tensor_add(den[:], den[:], counts[:, qt0:qt0 + 2])
                    rr = small.tile([128, 2], f32, tag="rr")
                    nc.vector.reciprocal(rr[:], den[:])
                    a0 = small.tile([128, 128], f32, tag="a0")
                    a1 = small.tile([128, 256], f32, tag="a1", bufs=2)
                    nc.vector.tensor_scalar_mul(a0[:], e0[:], rr[:, 0:1])
                    nc.vector.tensor_scalar_mul(a1[:], e1[:], rr[:, 1:2])
                    atb = small.tile([128, 3, 128], bf, tag="atb", bufs=2)
                    for i, ablk in enumerate((a0[:], a1[:, 0:128],
                                              a1[:, 128:256])):
                        atp = psum.tile([128, 128], f32, tag="ps")
                        nc.tensor.transpose(atp[:], ablk, ident)
                        nc.scalar.copy(atb[:, i, :], atp[:])
                    c0p = psum.tile([128, 128], f32, tag="ps")
                    nc.tensor.matmul(c0p[:], v_sb[:, kh, 2 * s, :], atb[:, 0, :],
                                     start=True, stop=True)
                    c1p = psum.tile([128, 128], f32, tag="ps")
                    nc.tensor.matmul(c1p[:], v_sb[:, kh, 2 * s, :], atb[:, 1, :],
                                     start=True, stop=False)
                    nc.tensor.matmul(c1p[:], v_sb[:, kh, 2 * s + 1, :],
                                     atb[:, 2, :], start=False, stop=True)
                    nc.scalar.copy(ctx[:, h, base:base + 128], c0p[:])
                    nc.scalar.copy(ctx[:, h, base + 128:base + 256], c1p[:])

                # ---- wo partial (this chunk) + bf16 AllReduce ----
                ch = s
                x1p = acts.tile([128, NDT, TC], bf, tag="x1p", bufs=1)
                for fp in range(NDT // 2):
                    ps = psum.tile([128, 2, TC], f32, tag="ps")
                    for sub in range(2):
                        ft = fp * 2 + sub
                        for et in range(EQT):
                            c0 = (ft * EQT + et) * 128
                            nc.tensor.matmul(ps[:, sub, :], wo[:, c0:c0 + 128],
                                             ctx[:, et, cols],
                                             start=(et == 0),
                                             stop=(et == EQT - 1))
                    if fp % 2 == 0:
                        nc.vector.tensor_copy(x1p[:, fp * 2:fp * 2 + 2, :], ps[:])
                    else:
                        nc.scalar.copy(x1p[:, fp * 2:fp * 2 + 2, :], ps[:])
                ar_in = dram.tile([128, NDT * TC], bf, tag=f"ar_in{ch}",
                                  name=f"ar_in{ch}")
                ar_out = dram.tile(
                    [128, NDT * TC], bf, tag=f"ar_out{ch}",
                    name=f"ar_out{ch}",
                    addr_space="Shared" if TP > 4 else "Local")
                nc.sync.dma_start(ar_in[:], x1p[:])
                nc.gpsimd.collective_compute(
                    "AllReduce", mybir.AluOpType.add, replica_groups=rg,
                    ins=[ar_in.opt()], outs=[ar_out.opt()])
                x1s = acts.tile([128, NDT, TC], bf, tag="x1s",
                                name=f"x1s_{ch}", bufs=1)
                nc.sync.dma_start(
                    x1s[:], ar_out[:].rearrange("p (a b) -> p a b", b=TC))
                nc.sync.dma_start(
                    x1s_out.rearrange("p (a b) -> p a b", b=T_G)[:, :, cols],
                    ar_out[:].rearrange("p (a b) -> p a b", b=TC))
                x1 = acts.tile([128, NDT, TC], bf, tag=f"x1_{ch}",
                               name=f"x1_{ch}")
                nc.vector.tensor_add(x1[:], x1s[:], xT[:, :, cols])
                x1_ch.append(x1)
                # norm2 for this chunk
                rbc2, _ = rms_stats(x1, TC, slice(0, TC))
                h2 = acts.tile([128, NDT, TC], bf, tag=f"h2_{ch}",
                               name=f"h2_{ch}")
                for qd in range(4):
                    dsl_ = slice(qd * 4, qd * 4 + 4)
                    nc.vector.tensor_tensor(
                        h2[:, dsl_, :], x1[:, dsl_, :],
                        rbc2[:, None, :].broadcast_to([128, 4, TC]), Alu.mult)
                h2_ch.append(h2)

            # ---- MLP per chunk: gate/up -> down(flipped) -> ReduceScatter ----
            m_ch = [acts.tile([128, FGT, TC], bf, tag=f"m_{ch}", name=f"m_{ch}")
                    for ch in range(S_G)]
            for ch in range(S_G):
                for ft in range(FGT):
                    gsl = wslab.tile([128, NDT * 128], bf, tag="wslab")
                    nc.gpsimd.dma_start(
                        gsl[:], wgT_in[:, ft * NDT * 128:(ft + 1) * NDT * 128])
                    usl = wslab.tile([128, NDT * 128], bf, tag="wslab")
                    nc.gpsimd.dma_start(
                        usl[:], wuT_in[:, ft * NDT * 128:(ft + 1) * NDT * 128])
                    gp = psum.tile([128, TC], f32, tag="ps")
                    up = psum.tile([128, TC], f32, tag="ps")
                    for dti in range(NDT):
                        nc.tensor.matmul(gp[:], gsl[:, dti * 128:(dti + 1) * 128],
                                         h2_ch[ch][:, dti, :],
                                         start=(dti == 0), stop=(dti == NDT - 1))
                    for dti in range(NDT):
                        nc.tensor.matmul(up[:], usl[:, dti * 128:(dti + 1) * 128],
                                         h2_ch[ch][:, dti, :],
                                         start=(dti == 0), stop=(dti == NDT - 1))
                    gs = small.tile([128, TC], f32, tag="gs", bufs=2)
                    nc.scalar.activation(gs[:], gp[:], Act.Silu)
                    nc.vector.tensor_mul(m_ch[ch][:, ft, :], gs[:], up[:])

                # down flipped: lhsT = m tiles, rhs = w_down column slabs
                for part in range(2):
                    rs_in = dram.tile([128, D], bf, tag=f"rs_in{ch}_{part}",
                                      name=f"rs_in{ch}_{part}")
                    tsl = slice(part * 128, part * 128 + 128)
                    for och in range(4):
                        if WD_RES:
                            dsl = wd_res.rearrange(
                                "p (a b) -> p a b", b=D)[:, :, och * 512:
                                                         (och + 1) * 512]
                        else:
                            dslt = wslab.tile([128, FGT, 512], bf,
                                              tag="wdslab", bufs=2)
                            nc.gpsimd.dma_start(
                                dslt[:],
                                wdT_in.rearrange("p (a b) -> p a b", b=D)
                                [:, :, och * 512:(och + 1) * 512])
                            dsl = dslt[:]
                        ps = psum.tile([128, 512], f32, tag="ps")
                        for ft in range(FGT):
                            nc.tensor.matmul(
                                ps[:], m_ch[ch][:, ft, tsl],
                                dsl[:, ft, :],
                                start=(ft == 0), stop=(ft == FGT - 1))
                        dr = small.tile([128, 512], bf, tag="x2dr", bufs=2)
                        if och % 2 == 0:
                            nc.vector.tensor_copy(dr[:], ps[:])
                        else:
                            nc.scalar.copy(dr[:], ps[:])
                        nc.sync.dma_start(
                            rs_in[:, och * 512:(och + 1) * 512], dr[:])
                    rs_out = dram.tile([128 // TP, D], bf,
                                       tag=f"rs_out{ch}_{part}",
                                       name=f"rs_out{ch}_{part}")
                    nc.gpsimd.collective_compute(
                        "ReduceScatter", mybir.AluOpType.add, replica_groups=rg,
                        ins=[rs_in.opt()], outs=[rs_out.opt()])
                    orow = (ch * 2 + part) * RS_OUT
                    nc.sync.dma_start(out_ap[orow:orow + RS_OUT, :], rs_out[:])

    nc.compile()
    return nc


def _host_prep(hidden_states, router_w, wq, wk, wv, wo, w_gate, w_up, w_down,
               ln1_w, ln2_w):
    x0 = np.asarray(hidden_states, np.float32)
    router_w = np.asarray(router_w, np.float32)
    rw = (x0.reshape(B * S, D) @ router_w.reshape(D)).reshape(B, S)
    k_cap = max(1, int(GAMMA * S))
    sel_idx, counts, rw_sel, xsel = [], [], [], []
    for b in range(B):
        thr = np.partition(rw[b], S - k_cap)[S - k_cap]
        idx = np.nonzero(rw[b] >= thr)[0]
        sel_idx.append(idx)
        counts.append((idx - np.arange(len(idx))).astype(np.float32))
        rw_sel.append(rw[b, idx])
        xsel.append(x0[b, idx])

    inv = 1.0 / (THETA ** (np.arange(0, HD, 2, dtype=np.float32) / HD))
    sgn = np.concatenate([-np.ones(64, np.float32), np.ones(64, np.float32)])
    cos_l, sin_l = [], []
    for b in range(B):
        fr = sel_idx[b].astype(np.float32)[:, None] * inv[None, :]
        emb = np.concatenate([fr, fr], axis=1)
        cos_l.append(np.cos(emb).T)
        sin_l.append((np.sin(emb) * sgn[None, :]).T)

    scale = np.float32(1.0 / np.sqrt(HD))
    xsel_all = np.concatenate(xsel, axis=0)             # [512, 2048]
    cos_all = np.concatenate(cos_l, axis=1)
    sin_all = np.concatenate(sin_l, axis=1)
    counts_all = np.concatenate(counts)

    cmask = np.triu(np.full((128, 128), -60000.0, np.float32), 1)
    pswap = np.zeros((128, 128), np.float32)
    pswap[(np.arange(128) + 64) % 128, np.arange(128)] = 1.0
    ones = np.ones((128, 128), np.float32)
    ident = np.eye(128, dtype=np.float32)

    ln1 = np.asarray(ln1_w, np.float32)
    ln2 = np.asarray(ln2_w, np.float32)
    wq_f = np.asarray(wq, np.float32) * ln1[None, :]
    wk_f = np.asarray(wk, np.float32) * ln1[None, :]
    wv_f = np.asarray(wv, np.float32) * ln1[None, :]
    wo_f = np.asarray(wo, np.float32)
    wg_f = np.asarray(w_gate, np.float32) * ln2[None, :]
    wu_f = np.asarray(w_up, np.float32) * ln2[None, :]
    wd_f = np.asarray(w_down, np.float32)

    in_maps = []
    for c in range(NCORES):
        g, r = c // TP, c % TP
        tokens = np.arange(g * T_G, (g + 1) * T_G)
        cos_g = cos_all[:, tokens]
        sin_g = sin_all[:, tokens]
        counts_g = counts_all[tokens]
        xsel_g = xsel_all[tokens]
        inB = np.concatenate([
            _pack_kxn(xsel_g.T.astype(np.float32)).astype(BF16),
            _pack_lhsT(wq_f[r * EQ:(r + 1) * EQ].T).astype(BF16),
            _pack_lhsT(wk_f[r * EK:(r + 1) * EK].T).astype(BF16),
            _pack_lhsT(wv_f[r * EK:(r + 1) * EK].T).astype(BF16),
            (cos_g * scale).astype(BF16),
            (sin_g * scale).astype(BF16),
            cos_g.astype(BF16),
            sin_g.astype(BF16),
            pswap.astype(BF16),
        ], axis=1)
        inC = _pack_lhsT(wo_f.T[r * EQ:(r + 1) * EQ]).astype(BF16)
        inD = np.concatenate([
            np.ascontiguousarray(
                counts_g.reshape(TT_G, 128).T).astype(np.float32),
            cmask, ones, ident,
        ], axis=1)
        m = {
            "inB": np.ascontiguousarray(inB),
            "inC": np.ascontiguousarray(inC),
            "inD": np.ascontiguousarray(inD),
            "wgT": _pack_lhsT(wg_f[r * FG:(r + 1) * FG].T).astype(BF16),
            "wuT": _pack_lhsT(wu_f[r * FG:(r + 1) * FG].T).astype(BF16),
            "wdT": _pack_kxn(wd_f.T[r * FG:(r + 1) * FG]).astype(BF16),
        }
        in_maps.append(m)
    return x0, sel_idx, rw_sel, xsel_all, in_maps


def kernel(hidden_states, router_w, wq, bq, wk, bk, wv, bv, wo,
           w_gate, w_up, w_down, ln1_w, ln2_w):
    global _NC
    from concourse import bass_utils

    x0, sel_idx, rw_sel, xsel_all, in_maps = _host_prep(
        hidden_states, router_w, wq, wk, wv, wo, w_gate, w_up, w_down,
        ln1_w, ln2_w)

    if _NC is None:
        _NC = _build_nc()

    res = bass_utils.run_bass_kernel_spmd(
        _NC, in_maps, core_ids=list(range(NCORES)),
        **_RUN_STATE.get("run_kwargs", {}))
    _RUN_STATE["last_results"] = res

    # x1 (pre-MLP residual stream) from dumped AR results, one core per group
    x1sT = np.empty((D, TTOT), np.float32)
    for g in range(G):
        xv = res.results[g * TP]["x1s_out"].astype(np.float32)
        xv = xv.reshape(128, NDT, T_G).transpose(1, 0, 2).reshape(D, T_G)
        x1sT[:, g * T_G:(g + 1) * T_G] = xv
    x1_full = x1sT.T + xsel_all                         # [512, 2048]

    # x2 (mlp output) from token-major RS shards
    x2 = np.empty((TTOT, D), np.float32)
    for c in range(NCORES):
        g, r = c // TP, c % TP
        sh = res.results[c]["out_shard"].astype(np.float32)
        for ch in range(S_G):
            for part in range(2):
                orow = (ch * 2 + part) * RS_OUT
                t0 = g * T_G + ch * 256 + part * 128 + r * RS_OUT
                x2[t0:t0 + RS_OUT, :] = sh[orow:orow + RS_OUT, :]
    block_out = x1_full + x2

    final = x0.copy()
    for b in range(B):
        rows = block_out[b * NSEL:(b + 1) * NSEL] * rw_sel[b][:, None]
        final[b, sel_idx[b]] = rows
    return final.astype(np.float32)


# revision 47
# speedup vs baseline: 1.0412x; 1.0248x over previous
"""MoD (mixture-of-depths) Qwen2 block — Trainium2 Bass kernel, 8 NeuronCores.

Structure: only 256 of 2048 tokens per sequence are selected (gamma=0.125);
non-selected tokens are zeroed, so their K/V are exactly zero and contribute
exp(0)=1 per causally-visible zero key to each softmax denominator.  The block
collapses to dense compute over the gathered tokens plus a per-query count
correction (count_i = pos_i - rank_i); causality on gathered indices is plain
lower-triangular.

Parallelization: TP-way tensor parallel within groups of TP cores, sequences
data-parallel across the 8/TP groups.  bf16 matmuls (fp32 PSUM), feature-major
activations.  RMSNorm#1 is folded into the RoPE tables / a transposed V scale,
so QKV never waits on the norm.  bf16 AllReduce after wo per chunk; the down
projection runs "flipped" (stationary = m tiles, moving = w_down columns,
token-major output) feeding bf16 ReduceScatters split along tokens; the x1
residual is applied on the host from a dumped AR result.
"""
import numpy as np
import ml_dtypes

# ---- static problem config (hardcoded per spec) ----
B, S, D = 2, 2048, 2048
HQ, HKV, HD = 16, 8, 128
FF = 8192
GAMMA = 0.125
EPS = 1e-6
THETA = 10000.0
NCORES = 8

TP = 8                       # tensor-parallel degree (cores per group)
G = NCORES // TP             # sequence-parallel groups
NSEL = 256                   # selected tokens per sequence
TTOT = B * NSEL
T_G = TTOT // G              # tokens per group
TT_G = T_G // 128
S_G = B // G                 # sequences (= AR chunks) per group
TC = 256                     # tokens per chunk (one sequence)
NDT = D // 128
EQ = HQ * HD // TP
EQT = EQ // 128
EK = HKV * HD // TP
EKT = EK // 128
FG = FF // TP
FGT = FG // 128
RS_OUT = 128 // TP           # token rows per core per RS part

BF16 = ml_dtypes.bfloat16

_NC = None
_RUN_STATE = {}


def _pack_kxn(a):
    """[K, N] -> [128, (K/128)*N]; k-tile-major, full-width N chunks."""
    a = np.ascontiguousarray(a)
    K, N = a.shape
    return np.ascontiguousarray(
        a.reshape(K // 128, 128, N).transpose(1, 0, 2).reshape(128, -1))


def _pack_lhsT(a):
    """[K, M] -> [128, (M/128)*(K/128)*128]; cols of tile (mt, kt) start at
    (mt*KT + kt)*128."""
    a = np.ascontiguousarray(a)
    K, M = a.shape
    KT, MT = K // 128, M // 128
    return np.ascontiguousarray(
        a.reshape(KT, 128, MT, 128).transpose(1, 2, 0, 3).reshape(128, MT * KT * 128)
    )


def _build_nc():
    import concourse.mybir as mybir
    import concourse.tile as tile
    from concourse import bacc

    dt = mybir.dt
    f32, bf = dt.float32, dt.bfloat16
    Alu = mybir.AluOpType
    Act = mybir.ActivationFunctionType

    nc = bacc.Bacc("TRN2", target_bir_lowering=False, debug=False,
                   enable_asserts=False, num_devices=NCORES)

    def din(name, shape, dtype=f32):
        return nc.dram_tensor(name, list(shape), dtype, kind="ExternalInput").ap()

    # packed bf16 input: xT | wq | wk | wv | cosq|sinq|cosk|sink | pswap
    NB_XT = NDT * T_G
    NB_WQ = EQT * NDT * 128
    NB_WK = EKT * NDT * 128
    NB = NB_XT + NB_WQ + 2 * NB_WK + 4 * T_G + 128
    inB_in = din("inB", [128, NB], bf)
    # packed bf16 input 2: wo
    NC_WO = NDT * EQT * 128
    inC_in = din("inC", [128, NC_WO], bf)
    # packed f32 input: counts | cmask | ones | ident
    ND_ = TT_G + 3 * 128
    inD_in = din("inD", [128, ND_])
    wgT_in = din("wgT", [128, FGT * NDT * 128], bf)
    wuT_in = din("wuT", [128, FGT * NDT * 128], bf)
    wdT_in = din("wdT", [128, FGT * D], bf)
    WD_RES = TP >= 8

    out_ap = nc.dram_tensor("out_shard", [S_G * 2 * RS_OUT, D], bf,
                            kind="ExternalOutput").ap()
    x1s_out = nc.dram_tensor("x1s_out", [128, NDT * T_G], bf,
                             kind="ExternalOutput").ap()

    rg = [list(range(g * TP, (g + 1) * TP)) for g in range(G)]

    with tile.TileContext(nc) as tc:
        with (
            tc.tile_pool(name="const", bufs=1) as constp,
            tc.tile_pool(name="wres", bufs=1) as wres,
            tc.tile_pool(name="acts", bufs=1) as acts,
            tc.tile_pool(name="wslab", bufs=3) as wslab,
            tc.tile_pool(name="small", bufs=3) as small,
            tc.tile_pool(name="psum", bufs=5, space="PSUM") as psum,
            tc.tile_pool(name="dram", bufs=1, space="DRAM") as dram,
        ):
            # ---- early sync barrier: a tiny AllReduce absorbs launch skew
            # while phase-1 compute runs (collectives ride TOPSP/SDMA only)
            bsync = constp.tile([1, 16], bf, tag="bsync")
            nc.vector.memset(bsync[:], 0.0)
            b_in = dram.tile([1, 16], bf, tag="b_in")
            b_out = dram.tile([1, 16], bf, tag="b_out",
                              addr_space="Shared" if TP > 4 else "Local")
            nc.sync.dma_start(b_in[:], bsync[:])
            nc.gpsimd.collective_compute(
                "AllReduce", mybir.AluOpType.add, replica_groups=rg,
                ins=[b_in.opt()], outs=[b_out.opt()])

            # ---- packed input loads (few DMA issues; early regions first) ----
            inB = acts.tile([128, NB], bf, tag="inB")
            nc.sync.dma_start(inB[:, 0:NB_XT // 2], inB_in[:, 0:NB_XT // 2])
            nc.sync.dma_start(inB[:, NB_XT // 2:NB_XT + NB_WQ],
                              inB_in[:, NB_XT // 2:NB_XT + NB_WQ])
            nc.sync.dma_start(inB[:, NB_XT + NB_WQ:NB], inB_in[:, NB_XT + NB_WQ:NB])
            inD = constp.tile([128, ND_], f32, tag="inD")
            nc.sync.dma_start(inD[:], inD_in)
            inC = wres.tile([128, NC_WO], bf, tag="inC")
            nc.sync.dma_start(inC[:], inC_in)
            if WD_RES:
                wd_res = wres.tile([128, FGT * D], bf, tag="wd_res")
                nc.sync.dma_start(wd_res[:], wdT_in)

            xT = inB[:, 0:NB_XT].rearrange("p (a b) -> p a b", b=T_G)
            wq = inB[:, NB_XT:NB_XT + NB_WQ]
            wk = inB[:, NB_XT + NB_WQ:NB_XT + NB_WQ + NB_WK]
            wv = inB[:, NB_XT + NB_WQ + NB_WK:NB_XT + NB_WQ + 2 * NB_WK]
            cbase = NB_XT + NB_WQ + 2 * NB_WK
            cosq = inB[:, cbase:cbase + T_G]
            sinq = inB[:, cbase + T_G:cbase + 2 * T_G]
            cosk = inB[:, cbase + 2 * T_G:cbase + 3 * T_G]
            sink = inB[:, cbase + 3 * T_G:cbase + 4 * T_G]
            pswap = inB[:, cbase + 4 * T_G:cbase + 4 * T_G + 128]
            wo = inC[:, 0:NC_WO]
            counts = inD[:, 0:TT_G]
            cmask = inD[:, TT_G:TT_G + 128]
            ones = inD[:, TT_G + 128:TT_G + 256]
            ident = inD[:, TT_G + 256:TT_G + 384]
            eps_sb = constp.tile([1, 1], f32, tag="eps")
            nc.vector.memset(eps_sb[:], EPS)
            ones_bf = constp.tile([128, 1], bf, tag="ones_bf")
            nc.vector.memset(ones_bf[:], 1.0)

            # ---- rmsnorm stats: rbc [128, n] psum with rows = rinv[t] ----
            def rms_stats(x3, n, ncols):
                msum = psum.tile([1, n], f32, tag="msum", bufs=1)
                for hh in range(2):
                    hsl = slice(hh * NDT // 2, (hh + 1) * NDT // 2)
                    sqa = small.tile([128, NDT // 2, n], bf, tag="sqa", bufs=2)
                    nc.vector.tensor_mul(sqa[:], x3[:, hsl, ncols],
                                         x3[:, hsl, ncols])
                    for dd in range(NDT // 2):
                        dti = hh * NDT // 2 + dd
                        nc.tensor.matmul(msum[:], ones_bf[:], sqa[:, dd, :],
                                         start=(dti == 0),
                                         stop=(dti == NDT - 1))
                rinv = small.tile([1, n], f32, tag="rinv")
                nc.scalar.activation(rinv[:], msum[:], Act.Abs_reciprocal_sqrt,
                                     bias=eps_sb[:], scale=1.0 / D)
                rbc_ps = psum.tile([128, n], f32, tag="rbc", bufs=1)
                nc.tensor.matmul(rbc_ps[:], ones[0:1], rinv[:], start=True,
                                 stop=True)
                rbc = small.tile([128, n], bf, tag="rbc_sb", bufs=2)
                nc.scalar.copy(rbc[:], rbc_ps[:])
                return rbc, rinv

            # ---- norm1 stats (norm folded into rope tables / V scale) ----
            rbc1, rinv1 = rms_stats(xT, T_G, slice(0, T_G))
            rinv_t = small.tile([128, TT_G], f32, tag="rinv_t", bufs=1)
            for tt in range(TT_G):
                rtp = psum.tile([128, 1], f32, tag="ps")
                nc.tensor.transpose(rtp[:], rinv1[0:1, tt * 128:(tt + 1) * 128],
                                    ident[0:1, 0:1])
                nc.vector.tensor_copy(rinv_t[:, tt:tt + 1], rtp[:])
            cq_s = acts.tile([128, T_G], bf, tag="cq_s")
            sq_s = acts.tile([128, T_G], bf, tag="sq_s")
            ck_s = acts.tile([128, T_G], bf, tag="ck_s")
            sk_s = acts.tile([128, T_G], bf, tag="sk_s")
            nc.vector.tensor_mul(cq_s[:], cosq, rbc1[:])
            nc.vector.tensor_mul(sq_s[:], sinq, rbc1[:])
            nc.vector.tensor_mul(ck_s[:], cosk, rbc1[:])
            nc.vector.tensor_mul(sk_s[:], sink, rbc1[:])

            # ---- QKV (all tokens, N=T_G) + full-width RoPE ----
            q_sb = acts.tile([128, EQT, T_G], bf, tag="q")
            k_sb = acts.tile([128, EKT, T_G], bf, tag="k")
            v_sb = acts.tile([128, EKT, TT_G, 128], bf, tag="v")
            ctx = acts.tile([128, EQT, T_G], bf, tag="ctx")

            def rope(ps, cos_t, sin_t, out2):
                raw = small.tile([128, T_G], bf, tag="rope_raw")
                nc.scalar.copy(raw[:], ps[:])
                rot = psum.tile([128, T_G], f32, tag="ps")
                nc.tensor.matmul(rot[:], pswap, raw[:], start=True, stop=True)
                t1 = small.tile([128, T_G], f32, tag="rope_t1", bufs=2)
                nc.vector.tensor_mul(t1[:], raw[:], cos_t)
                t2 = small.tile([128, T_G], f32, tag="rope_t2", bufs=2)
                nc.vector.tensor_mul(t2[:], rot[:], sin_t)
                nc.vector.tensor_add(out2, t1[:], t2[:])

            for et in range(EQT):
                ps = psum.tile([128, T_G], f32, tag="ps")
                for dti in range(NDT):
                    c0 = (et * NDT + dti) * 128
                    nc.tensor.matmul(ps[:], wq[:, c0:c0 + 128], xT[:, dti, :],
                                     start=(dti == 0), stop=(dti == NDT - 1))
                rope(ps, cq_s[:], sq_s[:], q_sb[:, et, :])
            for et in range(EKT):
                ps = psum.tile([128, T_G], f32, tag="ps")
                for dti in range(NDT):
                    c0 = (et * NDT + dti) * 128
                    nc.tensor.matmul(ps[:], wk[:, c0:c0 + 128], xT[:, dti, :],
                                     start=(dti == 0), stop=(dti == NDT - 1))
                rope(ps, ck_s[:], sk_s[:], k_sb[:, et, :])
            for kh in range(EKT):
                for tt in range(TT_G):
                    vp = psum.tile([128, 128], f32, tag="ps")
                    for dti in range(NDT):
                        c0 = (kh * NDT + dti) * 128
                        nc.tensor.matmul(
                            vp[:], xT[:, dti, tt * 128:(tt + 1) * 128],
                            wv[:, c0:c0 + 128],
                            start=(dti == 0), stop=(dti == NDT - 1))
                    nc.vector.tensor_scalar_mul(v_sb[:, kh, tt, :], vp[:],
                                                rinv_t[:, tt:tt + 1])

            x1_ch = []
            h2_ch = []
            for s in range(S_G):
                base = s * 256
                cols = slice(base, base + 256)
                # ---- attention per local q-head ----
                for h in range(EQT):
                    kh = h // 2
                    s0 = psum.tile([128, 128], f32, tag="ps")
                    nc.tensor.matmul(s0[:], q_sb[:, h, base:base + 128],
                                     k_sb[:, kh, base:base + 128],
                                     start=True, stop=True)
                    s1 = psum.tile([128, 256], f32, tag="ps")
                    nc.tensor.matmul(s1[:], q_sb[:, h, base + 128:base + 256],
                                     k_sb[:, kh, base:base + 256],
                                     start=True, stop=True)
                    nc.vector.tensor_add(s0[:], s0[:], cmask)
                    nc.vector.tensor_add(s1[:, 128:256], s1[:, 128:256], cmask)
                    e0 = small.tile([128, 128], f32, tag="e0")
                    e1 = small.tile([128, 256], f32, tag="e1", bufs=2)
                    den = small.tile([128, 2], f32, tag="den")
                    nc.scalar.activation(e0[:], s0[:], Act.Exp,
                                         accum_out=den[:, 0:1])
                    nc.scalar.activation(e1[:], s1[:], Act.Exp,
                                         accum_out=den[:, 1:2])
                    qt0 = 2 * s
                    nc.vector.tensor_add(den[:], den[:], counts[:, qt0:qt0 + 2])
                    rr = small.tile([128, 2], f32, tag="rr")
                    nc.vector.reciprocal(rr[:], den[:])
                    a0 = small.tile([128, 128], f32, tag="a0")
                    a1 = small.tile([128, 256], f32, tag="a1", bufs=2)
                    nc.vector.tensor_scalar_mul(a0[:], e0[:], rr[:, 0:1])
                    nc.vector.tensor_scalar_mul(a1[:], e1[:], rr[:, 1:2])
                    atb = small.tile([128, 3, 128], bf, tag="atb", bufs=2)
                    for i, ablk in enumerate((a0[:], a1[:, 0:128],
                                              a1[:, 128:256])):
                        atp = psum.tile([128, 128], f32, tag="ps")
                        nc.tensor.transpose(atp[:], ablk, ident)
                        nc.scalar.copy(atb[:, i, :], atp[:])
                    c0p = psum.tile([128, 128], f32, tag="ps")
                    nc.tensor.matmul(c0p[:], v_sb[:, kh, 2 * s, :], atb[:, 0, :],
                                     start=True, stop=True)
                    c1p = psum.tile([128, 128], f32, tag="ps")
                    nc.tensor.matmul(c1p[:], v_sb[:, kh, 2 * s, :], atb[:, 1, :],
                                     start=True, stop=False)
                    nc.tensor.matmul(c1p[:], v_sb[:, kh, 2 * s + 1, :],
                                     atb[:, 2, :], start=False, stop=True)
                    nc.scalar.copy(ctx[:, h, base:base + 128], c0p[:])
                    nc.scalar.copy(ctx[:, h, base + 128:base + 256], c1p[:])

                # ---- wo partial (this chunk) + bf16 AllReduce ----
                ch = s
                x1p = acts.tile([128, NDT, TC], bf, tag="x1p", bufs=1)
                for fp in range(NDT // 2):
                    ps = psum.tile([128, 2, TC], f32, tag="ps")
                    for sub in range(2):
                        ft = fp * 2 + sub
                        for et in range(EQT):
                            c0 = (ft * EQT + et) * 128
                            nc.tensor.matmul(ps[:, sub, :], wo[:, c0:c0 + 128],
                                             ctx[:, et, cols],
                                             start=(et == 0),
                                             stop=(et == EQT - 1))
                    if fp % 2 == 0:
                        nc.vector.tensor_copy(x1p[:, fp * 2:fp * 2 + 2, :], ps[:])
                    else:
                        nc.scalar.copy(x1p[:, fp * 2:fp * 2 + 2, :], ps[:])
                ar_in = dram.tile([128, NDT * TC], bf, tag=f"ar_in{ch}",
                                  name=f"ar_in{ch}")
                ar_out = dram.tile(
                    [128, NDT * TC], bf, tag=f"ar_out{ch}",
                    name=f"ar_out{ch}",
                    addr_space="Shared" if TP > 4 else "Local")
                nc.sync.dma_start(ar_in[:], x1p[:])
                nc.gpsimd.collective_compute(
                    "AllReduce", mybir.AluOpType.add, replica_groups=rg,
                    ins=[ar_in.opt()], outs=[ar_out.opt()])
                x1s = acts.tile([128, NDT, TC], bf, tag="x1s",
                                name=f"x1s_{ch}", bufs=1)
                nc.sync.dma_start(
                    x1s[:], ar_out[:].rearrange("p (a b) -> p a b", b=TC))
                nc.sync.dma_start(
                    x1s_out.rearrange("p (a b) -> p a b", b=T_G)[:, :, cols],
                    ar_out[:].rearrange("p (a b) -> p a b", b=TC))
                x1 = acts.tile([128, NDT, TC], bf, tag=f"x1_{ch}",
                               name=f"x1_{ch}")
                nc.vector.tensor_add(x1[:], x1s[:], xT[:, :, cols])
                x1_ch.append(x1)
                # norm2 for this chunk
                rbc2, _ = rms_stats(x1, TC, slice(0, TC))
                h2 = acts.tile([128, NDT, TC], bf, tag=f"h2_{ch}",
                               name=f"h2_{ch}")
                for qd in range(4):
                    dsl_ = slice(qd * 4, qd * 4 + 4)
                    nc.vector.tensor_tensor(
                        h2[:, dsl_, :], x1[:, dsl_, :],
                        rbc2[:, None, :].broadcast_to([128, 4, TC]), Alu.mult)
                h2_ch.append(h2)

            # ---- MLP per chunk: gate/up -> down(flipped) -> ReduceScatter ----
            m_ch = [acts.tile([128, FGT, TC], bf, tag=f"m_{ch}", name=f"m_{ch}")
                    for ch in range(S_G)]
            for ch in range(S_G):
                for ft in range(FGT):
                    gsl = wslab.tile([128, NDT * 128], bf, tag="wslab")
                    nc.gpsimd.dma_start(
                        gsl[:], wgT_in[:, ft * NDT * 128:(ft + 1) * NDT * 128])
                    usl = wslab.tile([128, NDT * 128], bf, tag="wslab")
                    nc.gpsimd.dma_start(
                        usl[:], wuT_in[:, ft * NDT * 128:(ft + 1) * NDT * 128])
                    gp = psum.tile([128, TC], f32, tag="ps")
                    up = psum.tile([128, TC], f32, tag="ps")
                    for dti in range(NDT):
                        nc.tensor.matmul(gp[:], gsl[:, dti * 128:(dti + 1) * 128],
                                         h2_ch[ch][:, dti, :],
                                         start=(dti == 0), stop=(dti == NDT - 1))
                    for dti in range(NDT):
                        nc.tensor.matmul(up[:], usl[:, dti * 128:(dti + 1) * 128],
                                         h2_ch[ch][:, dti, :],
                                         start=(dti == 0), stop=(dti == NDT - 1))
                    gs = small.tile([128, TC], f32, tag="gs", bufs=2)
                    nc.scalar.activation(gs[:], gp[:], Act.Silu)
                    nc.vector.tensor_mul(m_ch[ch][:, ft, :], gs[:], up[:])

                # down flipped: lhsT = m tiles, rhs = w_down column slabs
                for part in range(2):
                    rs_in = dram.tile([128, D], bf, tag=f"rs_in{ch}_{part}",
                                      name=f"rs_in{ch}_{part}")
                    tsl = slice(part * 128, part * 128 + 128)
                    for och in range(4):
                        if WD_RES:
                            dsl = wd_res.rearrange(
                                "p (a b) -> p a b", b=D)[:, :, och * 512:
                                                         (och + 1) * 512]
                        else:
                            dslt = wslab.tile([128, FGT, 512], bf,
                                              tag="wdslab", bufs=2)
                            nc.gpsimd.dma_start(
                                dslt[:],
                                wdT_in.rearrange("p (a b) -> p a b", b=D)
                                [:, :, och * 512:(och + 1) * 512])
                            dsl = dslt[:]
                        ps = psum.tile([128, 512], f32, tag="ps")
                        for ft in range(FGT):
                            nc.tensor.matmul(
                                ps[:], m_ch[ch][:, ft, tsl],
                                dsl[:, ft, :],
                                start=(ft == 0), stop=(ft == FGT - 1))
                        dr = small.tile([128, 512], bf, tag="x2dr", bufs=2)
                        if och % 2 == 0:
                            nc.vector.tensor_copy(dr[:], ps[:])
                        else:
                            nc.scalar.copy(dr[:], ps[:])
                        nc.sync.dma_start(
                            rs_in[:, och * 512:(och + 1) * 512], dr[:])
                    rs_out = dram.tile([128 // TP, D], bf,
                                       tag=f"rs_out{ch}_{part}",
                                       name=f"rs_out{ch}_{part}")
                    nc.gpsimd.collective_compute(
                        "ReduceScatter", mybir.AluOpType.add, replica_groups=rg,
                        ins=[rs_in.opt()], outs=[rs_out.opt()])
                    orow = (ch * 2 + part) * RS_OUT
                    nc.sync.dma_start(out_ap[orow:orow + RS_OUT, :], rs_out[:])

    nc.compile()
    return nc


def _host_prep(hidden_states, router_w, wq, wk, wv, wo, w_gate, w_up, w_down,
               ln1_w, ln2_w):
    x0 = np.asarray(hidden_states, np.float32)
    router_w = np.asarray(router_w, np.float32)
    rw = (x0.reshape(B * S, D) @ router_w.reshape(D)).reshape(B, S)
    k_cap = max(1, int(GAMMA * S))
    sel_idx, counts, rw_sel, xsel = [], [], [], []
    for b in range(B):
        thr = np.partition(rw[b], S - k_cap)[S - k_cap]
        idx = np.nonzero(rw[b] >= thr)[0]
        sel_idx.append(idx)
        counts.append((idx - np.arange(len(idx))).astype(np.float32))
        rw_sel.append(rw[b, idx])
        xsel.append(x0[b, idx])

    inv = 1.0 / (THETA ** (np.arange(0, HD, 2, dtype=np.float32) / HD))
    sgn = np.concatenate([-np.ones(64, np.float32), np.ones(64, np.float32)])
    cos_l, sin_l = [], []
    for b in range(B):
        fr = sel_idx[b].astype(np.float32)[:, None] * inv[None, :]
        emb = np.concatenate([fr, fr], axis=1)
        cos_l.append(np.cos(emb).T)
        sin_l.append((np.sin(emb) * sgn[None, :]).T)

    scale = np.float32(1.0 / np.sqrt(HD))
    xsel_all = np.concatenate(xsel, axis=0)             # [512, 2048]
    cos_all = np.concatenate(cos_l, axis=1)
    sin_all = np.concatenate(sin_l, axis=1)
    counts_all = np.concatenate(counts)

    cmask = np.triu(np.full((128, 128), -60000.0, np.float32), 1)
    pswap = np.zeros((128, 128), np.float32)
    pswap[(np.arange(128) + 64) % 128, np.arange(128)] = 1.0
    ones = np.ones((128, 128), np.float32)
    ident = np.eye(128, dtype=np.float32)

    ln1 = np.asarray(ln1_w, np.float32)
    ln2 = np.asarray(ln2_w, np.float32)
    wq_f = np.asarray(wq, np.float32) * ln1[None, :]
    wk_f = np.asarray(wk, np.float32) * ln1[None, :]
    wv_f = np.asarray(wv, np.float32) * ln1[None, :]
    wo_f = np.asarray(wo, np.float32)
    wg_f = np.asarray(w_gate, np.float32) * ln2[None, :]
    wu_f = np.asarray(w_up, np.float32) * ln2[None, :]
    wd_f = np.asarray(w_down, np.float32)

    in_maps = []
    for c in range(NCORES):
        g, r = c // TP, c % TP
        tokens = np.arange(g * T_G, (g + 1) * T_G)
        cos_g = cos_all[:, tokens]
        sin_g = sin_all[:, tokens]
        counts_g = counts_all[tokens]
        xsel_g = xsel_all[tokens]
        inB = np.concatenate([
            _pack_kxn(xsel_g.T.astype(np.float32)).astype(BF16),
            _pack_lhsT(wq_f[r * EQ:(r + 1) * EQ].T).astype(BF16),
            _pack_lhsT(wk_f[r * EK:(r + 1) * EK].T).astype(BF16),
            _pack_lhsT(wv_f[r * EK:(r + 1) * EK].T).astype(BF16),
            (cos_g * scale).astype(BF16),
            (sin_g * scale).astype(BF16),
            cos_g.astype(BF16),
            sin_g.astype(BF16),
            pswap.astype(BF16),
        ], axis=1)
        inC = _pack_lhsT(wo_f.T[r * EQ:(r + 1) * EQ]).astype(BF16)
        inD = np.concatenate([
            np.ascontiguousarray(
                counts_g.reshape(TT_G, 128).T).astype(np.float32),
            cmask, ones, ident,
        ], axis=1)
        m = {
            "inB": np.ascontiguousarray(inB),
            "inC": np.ascontiguousarray(inC),
            "inD": np.ascontiguousarray(inD),
            "wgT": _pack_lhsT(wg_f[r * FG:(r + 1) * FG].T).astype(BF16),
            "wuT": _pack_lhsT(wu_f[r * FG:(r + 1) * FG].T).astype(BF16),
            "wdT": _pack_kxn(wd_f.T[r * FG:(r + 1) * FG]).astype(BF16),
        }
        in_maps.append(m)
    return x0, sel_idx, rw_sel, xsel_all, in_maps


def kernel(hidden_states, router_w, wq, bq, wk, bk, wv, bv, wo,
           w_gate, w_up, w_down, ln1_w, ln2_w):
    global _NC
    from concourse import bass_utils

    x0, sel_idx, rw_sel, xsel_all, in_maps = _host_prep(
        hidden_states, router_w, wq, wk, wv, wo, w_gate, w_up, w_down,
        ln1_w, ln2_w)

    if _NC is None:
        _NC = _build_nc()

    res = bass_utils.run_bass_kernel_spmd(
        _NC, in_maps, core_ids=list(range(NCORES)),
        **_RUN_STATE.get("run_kwargs", {}))
    _RUN_STATE["last_results"] = res

    # x1 (pre-MLP residual stream) from dumped AR results, one core per group
    x1sT = np.empty((D, TTOT), np.float32)
    for g in range(G):
        xv = res.results[g * TP]["x1s_out"].astype(np.float32)
        xv = xv.reshape(128, NDT, T_G).transpose(1, 0, 2).reshape(D, T_G)
        x1sT[:, g * T_G:(g + 1) * T_G] = xv
    x1_full = x1sT.T + xsel_all                         # [512, 2048]

    # x2 (mlp output) from token-major RS shards
    x2 = np.empty((TTOT, D), np.float32)
    for c in range(NCORES):
        g, r = c // TP, c % TP
        sh = res.results[c]["out_shard"].astype(np.float32)
        for ch in range(S_G):
            for part in range(2):
                orow = (ch * 2 + part) * RS_OUT
                t0 = g * T_G + ch * 256 + part * 128 + r * RS_OUT
                x2[t0:t0 + RS_OUT, :] = sh[orow:orow + RS_OUT, :]
    block_out = x1_full + x2

    final = x0.copy()
    for b in range(B):
        rows = block_out[b * NSEL:(b + 1) * NSEL] * rw_sel[b][:, None]
        final[b, sel_idx[b]] = rows
    return final.astype(np.float32)
